# revision 1
# baseline (speedup 1.0000x reference)
"""Trainium2 Bass kernel for nn_CrossAttnBlockppTwoCams.

Sharding: 8 cores = 4 scene-groups x 2 pair-halves. Core (g, s) handles scene
group g (batch entries 4g..4g+3) and attention pairs {3s, 3s+1, 3s+2} of the 6
cross-camera pairs, all 4 heads each. With this split, each core produces two
COMPLETE output batch entries (4g+2s+0, 4g+2s+1) because the final 1x1-conv
channel blocks of those entries come exactly from this core's pairs.

Per core: GroupNorm (6 slot-inputs), q/k/v NIN projections (f32r matmuls),
6x4 = 12 attention units of [1024q x 1024k] with C=128 on partitions,
softmax via exp on ScalarE + ones-matmul partition reduce, final NIN into two
accumulators. Matmuls use float32r (TF32-like, 1 col/cycle); attention
probabilities are stored bf16; V^T is f32r via PE transpose.
"""
import sys
import os

sys.path.insert(0, '/opt/trn_rl_repo')

import numpy as np

B, C, HH, WW = 16, 128, 32, 32
HW = HH * WW
NH, COND, GROUPS, EPS = 4, 32, 32, 1e-6
SCALE = float(C) ** -0.5
PAIRS = [(0, 1), (1, 0), (2, 3), (3, 2), (0, 2), (2, 0)]  # (q cam, kv cam)

_PROG = None


def _build_nc(repeat=1):
    import concourse.bacc as bacc
    import concourse.tile as tile
    import concourse.mybir as mybir

    f32 = mybir.dt.float32
    f32r = mybir.dt.float32r
    bf16 = mybir.dt.bfloat16
    AF = mybir.ActivationFunctionType
    ALU = mybir.AluOpType
    X_AX = mybir.AxisListType.X

    nc = bacc.Bacc("TRN2", target_bir_lowering=False, debug=False, num_devices=8)

    d_xq = nc.dram_tensor("xq", [3, C, HW], f32, kind="ExternalInput")
    d_xkv = nc.dram_tensor("xkv", [3, C, HW], f32, kind="ExternalInput")
    d_qc = nc.dram_tensor("qc", [3, COND, HW], f32r, kind="ExternalInput")
    d_kc = nc.dram_tensor("kc", [3, COND, HW], f32r, kind="ExternalInput")
    d_gnv = nc.dram_tensor("gnv", [C, 2], f32, kind="ExternalInput")
    d_wA = nc.dram_tensor("wA", [C, 3 * 512], f32r, kind="ExternalInput")
    d_wB = nc.dram_tensor("wB", [COND, 3 * 512], f32r, kind="ExternalInput")
    d_bqkv = nc.dram_tensor("bqkv", [C, 12], f32, kind="ExternalInput")
    d_w3 = nc.dram_tensor("w3", [C, 12 * C], f32r, kind="ExternalInput")
    d_ident = nc.dram_tensor("ident", [C, C], f32, kind="ExternalInput")
    d_constr = nc.dram_tensor("constr", [C, C], f32r, kind="ExternalInput")
    d_gind = nc.dram_tensor("gind", [C, GROUPS], f32, kind="ExternalInput")
    d_gindT = nc.dram_tensor("gindT", [GROUPS, C], f32, kind="ExternalInput")
    d_out = nc.dram_tensor("out", [2, C, HW], f32, kind="ExternalOutput")

    with tile.TileContext(nc) as tc, nc.allow_low_precision(reason="f32r pipeline"):
        import contextlib
        ctx = contextlib.ExitStack()
        with ctx:
            cpool = ctx.enter_context(tc.tile_pool(name="consts", bufs=1))
            xpool = ctx.enter_context(tc.tile_pool(name="xp", bufs=2))
            hpool = ctx.enter_context(tc.tile_pool(name="hp", bufs=6))
            gns = ctx.enter_context(tc.tile_pool(name="gns", bufs=3))
            scr = ctx.enter_context(tc.tile_pool(name="scr", bufs=2))
            qpool = ctx.enter_context(tc.tile_pool(name="qp", bufs=6))
            kpool = ctx.enter_context(tc.tile_pool(name="kp", bufs=6))
            vpool = ctx.enter_context(tc.tile_pool(name="vp", bufs=2))
            vtpool = ctx.enter_context(tc.tile_pool(name="vtp", bufs=6))
            epool = ctx.enter_context(tc.tile_pool(name="ep", bufs=9))
            chpool = ctx.enter_context(tc.tile_pool(name="chp", bufs=4))
            opool = ctx.enter_context(tc.tile_pool(name="op", bufs=2))
            apool = ctx.enter_context(tc.tile_pool(name="ap", bufs=2))
            P1 = ctx.enter_context(tc.tile_pool(name="ps1", bufs=2, space="PSUM"))
            P2 = ctx.enter_context(tc.tile_pool(name="ps2", bufs=2, space="PSUM"))

            # ---- constants ----
            ident = cpool.tile([C, C], f32, tag="ident")
            nc.sync.dma_start(ident[:], d_ident[:])
            constr = cpool.tile([C, C], f32r, tag="constr")
            nc.sync.dma_start(constr[:], d_constr[:])
            ones_col = constr[:, 0:1]          # [128,1] ones (f32r)
            ones_row = constr[0:1, :]          # [1,128] ones (f32r)
            onesb = cpool.tile([C, 1], bf16, tag="onesb")
            nc.vector.tensor_copy(onesb[:], constr[:, 0:1].bitcast(f32))
            gind = cpool.tile([C, GROUPS], f32, tag="gind")
            nc.sync.dma_start(gind[:], d_gind[:])
            gindT = cpool.tile([GROUPS, C], f32, tag="gindT")
            nc.sync.dma_start(gindT[:], d_gindT[:])
            wA = cpool.tile([C, 3 * 512], f32r, tag="wA")
            nc.sync.dma_start(wA[:], d_wA[:])
            wB = cpool.tile([COND, 3 * 512], f32r, tag="wB")
            nc.sync.dma_start(wB[:], d_wB[:])
            bqkv = cpool.tile([C, 12], f32, tag="bqkv")
            nc.sync.dma_start(bqkv[:], d_bqkv[:])
            w3 = cpool.tile([C, 12 * C], f32r, tag="w3")
            nc.sync.dma_start(w3[:], d_w3[:])
            gnv = cpool.tile([C, 2], f32, tag="gnv")
            nc.sync.dma_start(gnv[:], d_gnv[:])
            epst = cpool.tile([GROUPS, 1], f32, tag="epst")
            nc.vector.memset(epst[:], EPS)
            qct = []
            kct = []
            for i in range(3):
                q_t = cpool.tile([COND, HW], f32r, tag=f"qc{i}")
                nc.sync.dma_start(q_t[:], d_qc[i])
                qct.append(q_t)
                k_t = cpool.tile([COND, HW], f32r, tag=f"kc{i}")
                nc.sync.dma_start(k_t[:], d_kc[i])
                kct.append(k_t)

            # ---- GroupNorm for the 6 slot-inputs ----
            def group_norm(src_ap):
                xt = xpool.tile([C, HW], f32, tag="xt")
                nc.sync.dma_start(xt[:], src_ap)
                stat = gns.tile([C, 2], f32, tag="stat")
                nc.vector.tensor_reduce(out=stat[:, 0:1], in_=xt[:], axis=X_AX, op=ALU.add)
                sq = scr.tile([C, HW], f32, tag="sq", bufs=1)
                nc.vector.tensor_tensor(out=sq[:], in0=xt[:], in1=xt[:], op=ALU.mult)
                nc.vector.tensor_reduce(out=stat[:, 1:2], in_=sq[:], axis=X_AX, op=ALU.add)
                ps_g = P2.tile([GROUPS, 2], f32, tag="tail")
                nc.tensor.matmul(ps_g[:], gind[:], stat[:], start=True, stop=True)
                mr = gns.tile([GROUPS, 2], f32, tag="mr")
                nc.vector.tensor_copy(mr[:, 0:1], ps_g[:, 0:1])
                mu2 = gns.tile([GROUPS, 1], f32, tag="mu2")
                nc.vector.tensor_tensor(out=mu2[:], in0=mr[:, 0:1], in1=mr[:, 0:1], op=ALU.mult)
                var = gns.tile([GROUPS, 1], f32, tag="var")
                nc.vector.tensor_tensor(out=var[:], in0=ps_g[:, 1:2], in1=mu2[:], op=ALU.subtract)
                lnv = gns.tile([GROUPS, 1], f32, tag="lnv")
                nc.scalar.activation(lnv[:], var[:], AF.Ln, bias=epst[:], scale=1.0)
                nc.scalar.activation(mr[:, 1:2], lnv[:], AF.Exp, scale=-0.5)
                ps_bc = P2.tile([C, 2], f32, tag="tail")
                nc.tensor.matmul(ps_bc[:], gindT[:], mr[:], start=True, stop=True)
                se = gns.tile([C, 1], f32, tag="se")
                nc.vector.tensor_tensor(out=se[:], in0=ps_bc[:, 1:2], in1=gnv[:, 0:1], op=ALU.mult)
                ms = gns.tile([C, 1], f32, tag="ms")
                nc.vector.tensor_tensor(out=ms[:], in0=ps_bc[:, 0:1], in1=se[:], op=ALU.mult)
                be = gns.tile([C, 1], f32, tag="be")
                nc.vector.tensor_tensor(out=be[:], in0=gnv[:, 1:2], in1=ms[:], op=ALU.subtract)
                ht = hpool.tile([C, HW], f32r, tag="ht")
                nc.vector.tensor_scalar(out=ht[:], in0=xt[:], scalar1=se[:], scalar2=be[:],
                                        op0=ALU.mult, op1=ALU.add)
                return ht

            for _rep in range(repeat):
              hq = [None] * 3
              hkv = [None] * 3
              for i in range(3):
                  hq[i] = group_norm(d_xq[i])
                  hkv[i] = group_norm(d_xkv[i])

              # ---- NIN projections + attention per slot ----
              def nin_head(proj, h, h_src, c_src, out_dt, pool, eng):
                  # proj: 0=q,1=k,2=v ; returns [128,1024] tile of dtype out_dt
                  wa = wA[:, proj * 512 + h * 128: proj * 512 + (h + 1) * 128]
                  wb = wB[:, proj * 512 + h * 128: proj * 512 + (h + 1) * 128]
                  ps = P1.tile([C, HW], f32, tag="mm")
                  for half in range(2):
                      fr = slice(half * 512, (half + 1) * 512)
                      nc.tensor.matmul(ps[:, fr], wa, h_src[:, fr], start=True, stop=False)
                      nc.tensor.matmul(ps[:, fr], wb, c_src[:, fr], start=False, stop=True)
                  t = pool.tile([C, HW], out_dt, tag="t")
                  bias = bqkv[:, proj * 4 + h: proj * 4 + h + 1]
                  if eng == "dve":
                      nc.vector.tensor_scalar_add(out=t[:], in0=ps[:], scalar1=bias)
                  else:
                      nc.scalar.activation(t[:], ps[:], AF.Identity, bias=bias, scale=1.0)
                  return t

              # ---- per-slot NIN + attention ----
              acc = [None, None]
              for i in range(3):
                  Qs = {}
                  Ks = {}
                  VTs = {}
                  for h in range(NH):
                      Qs[h] = nin_head(0, h, hq[i], qct[i], f32r, qpool, "dve" if h < 2 else "act")
                      Ks[h] = nin_head(1, h, hkv[i], kct[i], f32r, kpool, "dve" if h < 2 else "act")
                      vt_src = nin_head(2, h, hkv[i], kct[i], f32, vpool, "dve" if h < 2 else "act")
                      vt = vtpool.tile([C, HW], bf16, tag="vt")
                      for w in range(2):
                          ps_vt = P2.tile([C, 512], f32, tag="tail")
                          for blk in range(4):
                              kb = w * 4 + blk
                              nc.tensor.transpose(ps_vt[:, blk * 128:(blk + 1) * 128],
                                                  vt_src[:, kb * 128:(kb + 1) * 128], ident[:])
                          nc.vector.tensor_copy(vt[:, w * 512:(w + 1) * 512], ps_vt[:])
                      VTs[h] = vt

                  for h in range(NH):
                      u = 4 * i + h
                      q_t, k_t, vt_t = Qs[h], Ks[h], VTs[h]
                      E = []
                      for kt in range(8):
                          ps_s = P1.tile([C, HW], f32, tag="mm")
                          lhs = k_t[:, kt * 128:(kt + 1) * 128]
                          nc.tensor.matmul(ps_s[:, 0:512], lhs, q_t[:, 0:512], start=True, stop=True)
                          nc.tensor.matmul(ps_s[:, 512:1024], lhs, q_t[:, 512:1024], start=True, stop=True)
                          e_t = epool.tile([C, HW], bf16, tag="et")
                          nc.scalar.activation(e_t[:], ps_s[:], AF.Exp, scale=SCALE)
                          E.append(e_t)
                      ps_o = P2.tile([C, HW], f32, tag="tail")
                      for kt in range(8):
                          st, sp = kt == 0, kt == 7
                          lhs = vt_t[:, kt * 128:(kt + 1) * 128]
                          nc.tensor.matmul(ps_o[:, 0:512], lhs, E[kt][:, 0:512], start=st, stop=sp)
                          nc.tensor.matmul(ps_o[:, 512:1024], lhs, E[kt][:, 512:1024], start=st, stop=sp)
                      # denominator: 2-level bf16 pairwise tree + PE ones-reduce
                      quads = []
                      for t in range(2):
                          p0 = chpool.tile([C, HW], bf16, tag="chain")
                          nc.vector.tensor_tensor(out=p0[:], in0=E[4 * t][:], in1=E[4 * t + 1][:], op=ALU.add)
                          p1 = chpool.tile([C, HW], bf16, tag="chain")
                          nc.vector.tensor_tensor(out=p1[:], in0=E[4 * t + 2][:], in1=E[4 * t + 3][:], op=ALU.add)
                          nc.vector.tensor_tensor(out=p0[:], in0=p0[:], in1=p1[:], op=ALU.add)
                          quads.append(p0)
                      ps_d = P2.tile([1, HW], f32, tag="tail")
                      for half in range(2):
                          fr = slice(half * 512, (half + 1) * 512)
                          nc.tensor.matmul(ps_d[0:1, fr], onesb[:], quads[0][:, fr], start=True, stop=False)
                          nc.tensor.matmul(ps_d[0:1, fr], onesb[:], quads[1][:, fr], start=False, stop=True)
                      r_row = chpool.tile([1, HW], f32r, tag="chain")
                      nc.vector.reciprocal(out=r_row[:], in_=ps_d[0:1, :])
                      ps_b = P2.tile([C, HW], f32, tag="tail")
                      nc.tensor.matmul(ps_b[:, 0:512], ones_row, r_row[0:1, 0:512], start=True, stop=True)
                      nc.tensor.matmul(ps_b[:, 512:1024], ones_row, r_row[0:1, 512:1024], start=True, stop=True)
                      bsb = scr.tile([C, HW], f32, tag="bsb")
                      nc.scalar.copy(bsb[:], ps_b[:])
                      o_sb = opool.tile([C, HW], f32r, tag="osb")
                      nc.vector.tensor_tensor(out=o_sb[:], in0=ps_o[:], in1=bsb[:], op=ALU.mult)
                      ps_n = P2.tile([C, HW], f32, tag="tail")
                      w3u = w3[:, u * 128:(u + 1) * 128]
                      nc.tensor.matmul(ps_n[:, 0:512], w3u, o_sb[:, 0:512], start=True, stop=True)
                      nc.tensor.matmul(ps_n[:, 512:1024], w3u, o_sb[:, 512:1024], start=True, stop=True)
                      j = 0 if u < 6 else 1
                      if u % 6 == 0:
                          acc_t = apool.tile([C, HW], f32, tag="acc")
                          acc[j] = acc_t
                          nc.vector.tensor_copy(acc_t[:], ps_n[:])
                      else:
                          nc.vector.tensor_tensor(out=acc[j][:], in0=acc[j][:], in1=ps_n[:], op=ALU.add)
                      if u % 6 == 5:
                          nc.sync.dma_start(d_out[j], acc[j][:])

    nc.compile()
    return nc


def _get_prog(repeat=1):
    global _PROG
    if repeat != 1:
        return _build_nc(repeat=repeat)
    if _PROG is None:
        _PROG = _build_nc()
    return _PROG


def _make_in_maps(x, q_cond, k_a_cond, k_b_cond, gn_scale, gn_bias,
                  W0, b0, W1, b1, W2, b2, W3, b3):
    f4 = np.float32
    x = np.ascontiguousarray(x, f4).reshape(B, C, HW)
    q_cond = np.ascontiguousarray(q_cond, f4).reshape(B // 2, COND, HW)
    k_a = np.ascontiguousarray(k_a_cond, f4).reshape(B // 2, COND, HW)
    k_b = np.ascontiguousarray(k_b_cond, f4).reshape(B // 2, COND, HW)

    wA = np.concatenate([W0[:C], W1[:C], W2[:C]], axis=1).astype(f4)        # [128, 1536]
    wB = np.concatenate([W0[C:], W1[C:], W2[C:]], axis=1).astype(f4)        # [32, 1536]
    bqkv = np.stack([b0.reshape(NH, C), b1.reshape(NH, C), b2.reshape(NH, C)]) \
             .reshape(12, C).T.astype(f4).copy()                            # [128, 12]
    gnv = np.stack([gn_scale, gn_bias], axis=1).astype(f4)                  # [128, 2]
    ident = np.eye(C, dtype=f4)
    constr = np.zeros((C, C), f4)
    constr[:, 0] = 1.0
    constr[0, :] = 1.0
    gind = np.zeros((C, GROUPS), f4)
    for c in range(C):
        gind[c, c // (C // GROUPS)] = 1.0 / (C // GROUPS * HW)
    gindT = np.zeros((GROUPS, C), f4)
    for c in range(C):
        gindT[c // (C // GROUPS), c] = 1.0

    def qcs(b):
        return q_cond[b // 2]

    def kcs(b):
        return (k_a if b % 2 == 0 else k_b)[b // 2]

    in_maps = []
    for core in range(8):
        g, s = core // 2, core % 2
        plist = [3 * s + 0, 3 * s + 1, 3 * s + 2]
        xq = np.stack([x[4 * g + PAIRS[p][0]] for p in plist])
        xkv = np.stack([x[4 * g + PAIRS[p][1]] for p in plist])
        qc = np.stack([qcs(4 * g + PAIRS[p][0]) for p in plist])
        kc = np.stack([kcs(4 * g + PAIRS[p][1]) for p in plist])
        w3l = np.zeros((C, 12 * C), f4)
        for u in range(12):
            i, h = u // 4, u % 4
            f = 512 * plist[i] + 128 * h
            r = f % 768
            w3l[:, u * C:(u + 1) * C] = W3[r:r + C, :]
        in_maps.append({
            "xq": xq, "xkv": xkv, "qc": qc, "kc": kc, "gnv": gnv,
            "wA": wA, "wB": wB, "bqkv": bqkv, "w3": w3l,
            "ident": ident, "constr": constr, "gind": gind, "gindT": gindT,
        })
    return in_maps


def _assemble(results, x, b3):
    x = np.ascontiguousarray(x, np.float32)
    out = np.empty_like(x)
    for core in range(8):
        g, s = core // 2, core % 2
        o = results[core]["out"].reshape(2, C, HH, WW)
        for j in range(2):
            b = 4 * g + 2 * s + j
            out[b] = x[b] + o[j] + b3[:, None, None].astype(np.float32)
    return out


def kernel(**inputs):
    from concourse.bass_utils import run_bass_kernel_spmd
    nc = _get_prog()
    ins = {k: np.asarray(v) for k, v in inputs.items()}
    in_maps = _make_in_maps(**ins)
    res = run_bass_kernel_spmd(nc, in_maps, core_ids=list(range(8)))
    return _assemble(res.results, ins["x"], ins["b3"])

